# revision 51
# baseline (speedup 1.0000x reference)
"""Distributed multi-head attention (RoPE, non-causal) on 8 TRN2 NeuronCores.

Sharding: tensor-parallel over heads. Core c owns heads {2c, 2c+1}:
  - wq/wk/wv rows c*256:(c+1)*256 (output dim), x replicated (pre-shuffled),
  - attention computed locally per (batch, head),
  - per-(batch, quarter) AllGather of attention outputs (transposed, bf16),
  - each core computes output columns c*256:(c+1)*256 with its wo rows.

v2 structure (vs v1), ~893us vs 960us baseline (PE power-throttled to
13/16 = 1.95GHz on this box, so the matmul-stream floor is ~815us):
  - attn@v computed v-STATIONARY: lhsT = v block [k,hd], rhs = exp [k,q],
    so output lands pre-transposed [hd, q] with N=512 streams; no PE
    transposes and no LDWEIGHTS-bound N=129 matmuls.
  - softmax denominator: pairwise-tree sum of exp tiles over the k-block
    axis on DVE (bf16 4x mode), partition-reduced and broadcast in one
    all-ones matmul ([128,128] ones -> psd = denominator row in every
    partition); 1/x as ScalarE exp(-ln(x)) (the DVE reciprocal costs
    ~3.3us per tile and sat on the AllGather input path).
  - the ScalarE exp chain (16 x 686ns per unit) outruns the scores
    matmuls, so next-batch projection and prev-batch wo work is woven in
    at MATMUL granularity (generator micro-queue, pump(n) between scores)
    and the attn@v group runs after the weave, never waiting on exps.
    One-unit flush lag keeps the psd matmul off the DVE critical path.
  - collectives: one AllGather per half-batch (the ~30-45us per-op CC
    cost is latency-floor-dominated, so fewer+bigger is better), EXCEPT
    the last batch which ships four quarter AllGathers so its CC chain
    starts earlier and the serial tail is one quarter + one wo quarter.
  - host pre-shuffles x/weights to [128, ...] layouts so every DMA line
    is >= 8KB contiguous per partition; startup loads k-weights (first
    chunks) and x in pieces so the first matmul starts ~4us in.
  - RoPE multiplies read the projection PSUM directly (fp32), the
    add/sub pairs run in bf16 SBUF (DVE 4x mode).
  - fp8 DoubleRow for the projections was tried and REVERTED: weight
    quantization error (~3.6% RMS) passes straight through to the output
    (the attention average commutes through x @ dW), giving ~5.7e-2 rel
    err vs the 2e-2 gate, even though it ran at ~825us.
"""

import numpy as np
import ml_dtypes

B, S, D, H = 4, 2048, 2048, 16
HD = 128            # head dim
NCORES = 8
HPC = H // NCORES   # heads per core = 2
OSL = HPC * HD      # per-core o-slice = 256
ROWS = B * S        # 8192 flattened rows
DCH = D // 128      # 16 contraction chunks
SCH = 512           # seq chunk for projections
NBLK = ROWS // SCH  # 16 x column blocks
KB = S // 128       # 16 k-blocks per batch
QC = 512            # q chunk in attention
NQC = S // QC       # 4 quarters per batch
HB = S // 2         # half-batch column span of one AllGather unit
NU = B * 2          # 8 collective units (half batches)

BF16 = ml_dtypes.bfloat16
_NC_CACHE = None


def _build():
    import concourse.bass as bass  # noqa: F401
    import concourse.mybir as mybir
    import concourse.tile as tile
    from concourse import bacc

    fp32 = mybir.dt.float32
    bf16 = mybir.dt.bfloat16

    nc = bacc.Bacc(
        "TRN2",
        target_bir_lowering=False,
        debug=False,
        num_devices=NCORES,
    )

    # Pre-shuffled layouts (host): one contiguous >=8KB run per partition.
    xS = nc.declare_dram_parameter("xS", [128, NBLK, DCH, SCH], bf16,
                                   isOutput=False)
    wqS = nc.declare_dram_parameter("wqS", [128, 2, DCH, HD], bf16,
                                    isOutput=False)
    wkS = nc.declare_dram_parameter("wkS", [128, 2, DCH, HD], bf16,
                                    isOutput=False)
    wvS = nc.declare_dram_parameter("wvS", [128, DCH, OSL], bf16,
                                    isOutput=False)
    woS = nc.declare_dram_parameter("woS", [128, DCH, OSL], bf16,
                                    isOutput=False)
    cosd = nc.declare_dram_parameter("cosd", [128, S], bf16, isOutput=False)
    sind = nc.declare_dram_parameter("sind", [128, S], bf16, isOutput=False)
    outp = nc.declare_dram_parameter("out", [OSL, ROWS], fp32, isOutput=True)

    inv_sqrt_hd = 1.0 / float(np.sqrt(HD))

    with tile.TileContext(nc) as tc:
        with (
            tc.tile_pool(name="glob", bufs=1) as glob,
            tc.tile_pool(name="dram", bufs=1, space="DRAM") as dram,
            tc.tile_pool(name="qkv", bufs=2) as qkv,
            tc.tile_pool(name="xtp", bufs=2) as xtp,
            tc.tile_pool(name="attp", bufs=2) as attp,
            tc.tile_pool(name="treep", bufs=2) as treep,
            tc.tile_pool(name="ropep", bufs=2) as ropep,
            tc.tile_pool(name="rcpp", bufs=2) as rcpp,
            tc.tile_pool(name="atp", bufs=2) as atp,
            tc.tile_pool(name="gtp", bufs=2) as gtp,
            tc.tile_pool(name="otp", bufs=2) as otp,
            tc.tile_pool(name="psP", bufs=5, space="PSUM") as psP,
            tc.tile_pool(name="poP", bufs=2, space="PSUM") as poP,
            tc.tile_pool(name="psV", bufs=1, space="PSUM") as psV,
        ):
            ones128 = glob.tile([128, 128], bf16, name="ones128")
            nc.vector.memset(ones128[:], 1.0)

            wq_sb = glob.tile([128, 2, DCH, HD], bf16, name="wq_sb")
            wk_sb = glob.tile([128, 2, DCH, HD], bf16, name="wk_sb")
            wv_sb = glob.tile([128, DCH, OSL], bf16, name="wv_sb")
            wo_sb = glob.tile([128, DCH, OSL], bf16, name="wo_sb")
            cosb = glob.tile([128, S], bf16, name="cosb")
            sinb = glob.tile([128, S], bf16, name="sinb")
            # startup order matches the first chunks (k, then q, then v)
            nc.gpsimd.dma_start(wk_sb[:, 0, :, :], wkS[:, 0, :, :])
            xt00 = xtp.tile([128, DCH, SCH], bf16, name="xt00", tag="xt")
            for cq in range(4):
                nc.gpsimd.dma_start(xt00[:, cq * 4:(cq + 1) * 4, :],
                                    xS[:, 0, cq * 4:(cq + 1) * 4, :])
            nc.gpsimd.dma_start(wk_sb[:, 1, :, :], wkS[:, 1, :, :])
            nc.gpsimd.dma_start(wq_sb[:], wqS[:, :, :, :])
            nc.gpsimd.dma_start(wv_sb[:], wvS[:, :, :])
            nc.gpsimd.dma_start(cosb[:], cosd[:, :])
            nc.gpsimd.dma_start(sinb[:], sind[:, :])

            bounce = [dram.tile([OSL, HB], bf16, name=f"bounce{u}")
                      for u in range(NU)]
            gath = [dram.tile([NCORES * OSL, HB], bf16, addr_space="Shared",
                              name=f"gath{u}") for u in range(NU)]
            # the final batch ships as four quarter AllGathers so its
            # collectives start earlier and the serial tail is a quarter
            bounce_q = [dram.tile([OSL, QC], bf16, name=f"bounceq{j}")
                        for j in range(4)]
            gath_q = [dram.tile([NCORES * OSL, QC], bf16,
                                addr_space="Shared", name=f"gathq{j}")
                      for j in range(4)]

            def fetch_x(b, sc):
                xt = xtp.tile([128, DCH, SCH], bf16, name="xt", tag="xt")
                nc.gpsimd.dma_start(xt[:], xS[:, b * (S // SCH) + sc, :, :])
                return xt

            proj_result = {}

            def proj_gen(b, xt_first, parts="kqv"):
                """Generator issuing batch b's projection (the weight
                classes named in `parts`), yielding after every matmul so
                the pump can weave it between the attention scores
                matmuls (hides the ScalarE exp chain)."""
                if "k" in parts:
                    qt = qkv.tile([128, HPC, S], bf16, name="qt", tag="qt")
                    kt = qkv.tile([128, HPC, S], bf16, name="kt", tag="kt")
                    vt = qkv.tile([128, KB, HPC, HD], bf16, name="vt",
                                  tag="vt")
                    proj_result[b] = (qt, kt, vt)
                else:
                    qt, kt, vt = proj_result[b]
                xt, xt_next = xt_first, None

                for sc in range(S // SCH):
                    if sc > 0:
                        xt = xt_next
                    if sc + 1 < S // SCH:
                        xt_next = fetch_x(b, sc + 1)
                    sl = slice(sc * SCH, (sc + 1) * SCH)
                    cosr = cosb[:, sl]
                    sinr = sinb[:, sl]
                    def qk_chunk(w_sb, dstT, h):
                        ps = psP.tile([128, SCH], fp32, name="ps",
                                      tag="ps")
                        for c in range(DCH):
                            nc.tensor.matmul(
                                ps[:], w_sb[:, h, c, :],
                                xt[:, c, :],
                                start=(c == 0), stop=(c == DCH - 1))
                            yield
                        m1 = ropep.tile([128, SCH], bf16, name="m1",
                                        tag="m1")
                        m2 = ropep.tile([128, SCH], bf16, name="m2",
                                        tag="m2")
                        # m1 = [tr*cos ; ti*cos]; m2 swapped-halves =
                        # [ti*sin ; tr*sin] so the DVE add/sub uses
                        # equal SBUF base partitions (the PSUM operand
                        # may differ).
                        nc.vector.tensor_mul(m1[:], ps[:], cosr)
                        nc.vector.tensor_mul(
                            m2[0:64, :], ps[64:128, :], sinr[0:64, :])
                        nc.vector.tensor_mul(
                            m2[64:128, :], ps[0:64, :], sinr[64:128, :])
                        nc.vector.tensor_sub(
                            dstT[0:64, h, sl], m1[0:64, :], m2[0:64, :])
                        nc.vector.tensor_add(
                            dstT[64:128, h, sl], m2[64:128, :],
                            m1[64:128, :])

                    if "k" in parts:
                        yield from qk_chunk(wk_sb, kt, 0)
                        yield from qk_chunk(wk_sb, kt, 1)
                    if "q" in parts:
                        yield from qk_chunk(wq_sb, qt, 0)
                        yield from qk_chunk(wq_sb, qt, 1)
                    if "v" not in parts:
                        continue
                    for pair in range(2):
                        psv = psV.tile([128, 2, OSL], fp32, name="psv")
                        for j in range(2):
                            ssb = pair * 2 + j
                            for c in range(DCH):
                                nc.tensor.matmul(
                                    psv[:, j, :],
                                    xt[:, c, ssb * 128:(ssb + 1) * 128],
                                    wv_sb[:, c, :],
                                    start=(c == 0), stop=(c == DCH - 1))
                                yield
                        kb0 = sc * 4 + pair * 2
                        nc.vector.tensor_copy(
                            vt[:, kb0:kb0 + 2, :, :],
                            psv[:].rearrange("p s (h d) -> p s h d", h=HPC))

            def allgather(src, dst):
                nc.gpsimd.collective_compute(
                    "AllGather",
                    mybir.AluOpType.bypass,
                    ins=[src.opt()],
                    outs=[dst.opt()],
                    replica_groups=[list(range(NCORES))],
                )

            def rb_pair(src, rc_):
                """Issue the two gather-readback DMAs for 512 columns."""
                gh = []
                for dh in range(2):
                    g = gtp.tile([128, DCH // 2, QC], bf16, name="gt",
                                 tag="gt")
                    nc.sync.dma_start(
                        g[:],
                        src[dh * 1024:(dh + 1) * 1024,
                            rc_ * QC:(rc_ + 1) * QC]
                        .rearrange("(c p) n -> p c n", p=128))
                    gh.append(g)
                return gh

            def wo_cols(src, col0, ncols):
                """Generator: wo for `ncols` gathered columns starting at
                output column col0, reading gather tile `src`."""
                for rc_ in range(ncols // QC):
                    gh = rb_pair(src, rc_)
                    for oc in range(OSL // 128):
                        psw = psP.tile([128, QC], fp32, name="psw", tag="ps")
                        for c in range(DCH):
                            nc.tensor.matmul(
                                psw[:],
                                wo_sb[:, c, oc * 128:(oc + 1) * 128],
                                gh[c // 8][:, c % 8, :],
                                start=(c == 0), stop=(c == DCH - 1))
                            yield
                        out_t = otp.tile([128, QC], fp32, name="out_t")
                        nc.vector.tensor_copy(out_t[:], psw[:])
                        cc = col0 + rc_ * QC
                        nc.sync.dma_start(
                            outp[oc * 128:(oc + 1) * 128, cc:cc + QC],
                            out_t[:])

            def wo_half_gen(b, half):
                return wo_cols(gath[b * 2 + half], b * S + half * HB, HB)

            def wo_quarter_gen(qc):
                return wo_cols(gath_q[qc], (B - 1) * S + qc * QC, QC)

            # one-unit flush lag: normalize/ship unit i-1 during unit i so
            # the psd matmul never waits on the DVE/ScalarE chains
            pend = []

            def flush_one():
                if not pend:
                    return
                b, h, qc, po, dsum = pend.pop(0)
                psd = psP.tile([128, QC], fp32, name="psd", tag="ps")
                nc.tensor.matmul(psd[:], ones128[:], dsum[:],
                                 start=True, stop=True)
                # 1/x as exp(-ln(x)) on ScalarE: the DVE reciprocal op
                # costs ~3.3us per [128,512] tile and would sit on the
                # critical path to the AllGather input
                lnd = rcpp.tile([128, QC], fp32, name="lnd", tag="lnd",
                                bufs=1)
                nc.scalar.activation(lnd[:], psd[:],
                                     mybir.ActivationFunctionType.Ln)
                rcp = rcpp.tile([128, QC], fp32, name="rcp", tag="rcp")
                nc.scalar.activation(rcp[:], lnd[:],
                                     mybir.ActivationFunctionType.Exp,
                                     scale=-1.0)
                a_t = atp.tile([128, QC], bf16, name="a_t")
                nc.vector.tensor_mul(a_t[:], po[:], rcp[:])
                if b == B - 1:
                    nc.gpsimd.dma_start(
                        bounce_q[qc][h * HD:(h + 1) * HD, :], a_t[:])
                    if h == 1:
                        allgather(bounce_q[qc], gath_q[qc])
                else:
                    u = b * 2 + qc // 2
                    col0 = (qc % 2) * QC
                    nc.gpsimd.dma_start(
                        bounce[u][h * HD:(h + 1) * HD, col0:col0 + QC],
                        a_t[:])
                    if h == 1 and qc % 2 == 1:
                        allgather(bounce[u], gath[u])

            # the micro-queue: generators yielding once per issued matmul;
            # pump(n) issues up to n matmuls of weave-filler work
            micro = []

            def pump(n):
                k = 0
                while k < n and micro:
                    try:
                        next(micro[0])
                        k += 1
                    except StopIteration:
                        micro.pop(0)

            PUMP_RATE = {0: 5, 1: 5, 2: 5, 3: 2}

            def attn_unit(b, qt, kt, vt, h, qc):
                expT = attp.tile([128, KB, QC], bf16, name="expT")
                for kb in range(KB):
                    pss = psP.tile([128, QC], fp32, name="pss", tag="ps")
                    nc.tensor.matmul(
                        pss[:],
                        kt[:, h, kb * 128:(kb + 1) * 128],
                        qt[:, h, qc * QC:(qc + 1) * QC],
                        start=True, stop=True)
                    nc.scalar.activation(
                        expT[:, kb, :], pss[:],
                        mybir.ActivationFunctionType.Exp,
                        scale=inv_sqrt_hd)
                    # weave filler matmuls so the scores stream never
                    # outruns the 5-deep pss rotation / exp chain
                    pump(PUMP_RATE[b])
                # denominator: tree-sum the 16 k-blocks on DVE (bf16 4x)
                s1 = treep.tile([128, 4, QC], bf16, name="s1", tag="s1")
                s2 = treep.tile([128, 4, QC], bf16, name="s2", tag="s2")
                s4 = treep.tile([128, 2, QC], bf16, name="s4", tag="s4")
                dsum = treep.tile([128, QC], bf16, name="dsum", tag="ds")
                nc.vector.tensor_add(s1[:], expT[:, 0:4, :], expT[:, 4:8, :])
                nc.vector.tensor_add(s2[:], expT[:, 8:12, :],
                                     expT[:, 12:16, :])
                nc.vector.tensor_add(s1[:], s1[:], s2[:])
                nc.vector.tensor_add(s4[:], s1[:, 0:2, :], s1[:, 2:4, :])
                nc.vector.tensor_add(dsum[:], s4[:, 0, :], s4[:, 1, :])
                # normalize/ship the previous unit (one-unit lag)
                flush_one()
                po = poP.tile([128, QC], fp32, name="po", tag="po")
                for kb in range(KB):
                    nc.tensor.matmul(
                        po[:], vt[:, kb, h, 0:HD], expT[:, kb, :],
                        start=(kb == 0), stop=(kb == KB - 1))
                pend.append((b, h, qc, po, dsum))

            # ---------------- main schedule ---------------------------
            # proj(0) runs unfilled upfront; proj(b+1) and wo(b-1)
            # quarters fill the attention(b) units.
            # batch 0: only k/v must be complete before attention; its
            # q chunks weave into attention(0) as filler (x re-fetched)
            for _ in proj_gen(0, xt00, "kv"):
                pass
            micro.append(proj_gen(0, fetch_x(0, 0), "q"))
            pump(32)  # q head chunks for qc=0 must precede unit 0
            # wo weights are first needed ~300us in; load off the
            # startup critical path
            nc.gpsimd.dma_start(wo_sb[:], woS[:, :, :])

            for b in range(B):
                if b + 1 < B:
                    micro.append(proj_gen(b + 1, fetch_x(b + 1, 0)))
                qt, kt, vt = proj_result[b]
                # wo halves woven into batch b's units (appended to the
                # micro-queue at the given unit index); half1 of each
                # batch is deferred one extra batch to feed batch 3
                wo_sched = {}
                if b >= 2:
                    wo_sched[0] = [wo_half_gen(b - 2, 1)]
                if b >= 1:
                    wo_sched[2] = [wo_half_gen(b - 1, 0)]
                if b == B - 1:
                    wo_sched[4] = [wo_half_gen(b - 1, 1)]
                    # this batch's first quarter is gathered by unit 6
                    wo_sched[6] = [wo_quarter_gen(0)]
                units = [(h, qc) for qc in range(NQC) for h in range(HPC)]
                for ui, (h, qc) in enumerate(units):
                    micro.extend(wo_sched.get(ui, []))
                    attn_unit(b, qt, kt, vt, h, qc)
                if b + 1 < B:
                    # drain leftover weave work before the next batch
                    pump(10 ** 9)
            # tail: remaining quarters in AllGather-completion order
            flush_one()
            pump(10 ** 9)
            for qc in (1, 2, 3):
                for _ in wo_quarter_gen(qc):
                    pass

    nc.compile()
    return nc


def _shard_inputs(x, freqs_cos, freqs_sin, wq, wk, wv, wo):
    xf = np.asarray(x, dtype=np.float32).reshape(ROWS, D)
    xT = np.ascontiguousarray(xf.T).astype(BF16)  # [D, ROWS]
    # pre-shuffle: xS[p, blk, c, j] = xT[c*128+p, blk*512+j]
    xS = np.ascontiguousarray(
        xT.reshape(DCH, 128, NBLK, SCH).transpose(1, 2, 0, 3))
    fcT = np.asarray(freqs_cos, dtype=np.float32).T  # [64, S]
    fsT = np.asarray(freqs_sin, dtype=np.float32).T
    cosd = np.ascontiguousarray(np.concatenate([fcT, fcT], 0)).astype(BF16)
    sind = np.ascontiguousarray(np.concatenate([fsT, fsT], 0)).astype(BF16)
    # even indices (real half) then odd (imag half), per head
    perm = np.concatenate([np.arange(0, HD, 2), np.arange(1, HD, 2)])

    def shuf(wrows):  # [OSL, D] -> [128, DCH, OSL] bf16
        wT = np.ascontiguousarray(np.asarray(wrows, dtype=np.float32).T)
        return np.ascontiguousarray(
            wT.reshape(DCH, 128, OSL).transpose(1, 0, 2)).astype(BF16)

    def shufh(wrows):  # [OSL, D] -> [128, 2, DCH, HD] bf16 (head-major)
        wT = np.ascontiguousarray(np.asarray(wrows, dtype=np.float32).T)
        return np.ascontiguousarray(
            wT.reshape(DCH, 128, 2, HD).transpose(1, 2, 0, 3)).astype(BF16)

    in_maps = []
    for c in range(NCORES):
        rows = slice(c * OSL, (c + 1) * OSL)
        wq_c = np.asarray(wq)[rows].reshape(HPC, HD, D)[:, perm, :]
        wk_c = np.asarray(wk)[rows].reshape(HPC, HD, D)[:, perm, :]
        in_maps.append({
            "xS": xS,
            "wqS": shufh(wq_c.reshape(OSL, D)),
            "wkS": shufh(wk_c.reshape(OSL, D)),
            "wvS": shuf(np.asarray(wv)[rows]),
            "woS": shuf(np.asarray(wo)[rows]),
            "cosd": cosd,
            "sind": sind,
        })
    return in_maps


def run(inputs, trace=False, trace_cores=None):
    """Build (cached), run on 8 cores; returns (full_output, results)."""
    global _NC_CACHE
    from concourse.bass_utils import run_bass_kernel_spmd
    if _NC_CACHE is None:
        _NC_CACHE = _build()
    in_maps = _shard_inputs(**inputs)
    res = run_bass_kernel_spmd(
        _NC_CACHE, in_maps, core_ids=list(range(NCORES)), trace=trace,
        trace_cores=trace_cores)
    parts = [np.ascontiguousarray(
        np.asarray(res.results[c]["out"], dtype=np.float32).T)
        for c in range(NCORES)]
    full = np.concatenate(parts, axis=1).reshape(B, S, D)
    return full, res


def kernel(x, freqs_cos, freqs_sin, wq, wk, wv, wo):
    full, _ = run(dict(x=x, freqs_cos=freqs_cos, freqs_sin=freqs_sin,
                       wq=wq, wk=wk, wv=wv, wo=wo))
    return full


# revision 52
# speedup vs baseline: 1.0119x; 1.0119x over previous
"""Distributed multi-head attention (RoPE, non-causal) on 8 TRN2 NeuronCores.

Sharding: tensor-parallel over heads. Core c owns heads {2c, 2c+1}:
  - wq/wk/wv rows c*256:(c+1)*256 (output dim), x replicated (pre-shuffled),
  - attention computed locally per (batch, head),
  - per-(batch, quarter) AllGather of attention outputs (transposed, bf16),
  - each core computes output columns c*256:(c+1)*256 with its wo rows.

v2 structure (vs v1), ~893us vs 960us baseline (PE power-throttled to
13/16 = 1.95GHz on this box, so the matmul-stream floor is ~815us):
  - attn@v computed v-STATIONARY: lhsT = v block [k,hd], rhs = exp [k,q],
    so output lands pre-transposed [hd, q] with N=512 streams; no PE
    transposes and no LDWEIGHTS-bound N=129 matmuls.
  - softmax denominator: pairwise-tree sum of exp tiles over the k-block
    axis on DVE (bf16 4x mode), partition-reduced and broadcast in one
    all-ones matmul ([128,128] ones -> psd = denominator row in every
    partition); 1/x as ScalarE exp(-ln(x)) (the DVE reciprocal costs
    ~3.3us per tile and sat on the AllGather input path).
  - the ScalarE exp chain (16 x 686ns per unit) outruns the scores
    matmuls, so next-batch projection and prev-batch wo work is woven in
    at MATMUL granularity (generator micro-queue, pump(n) between scores)
    and the attn@v group runs after the weave, never waiting on exps.
    One-unit flush lag keeps the psd matmul off the DVE critical path.
  - collectives: one AllGather per half-batch (the ~30-45us per-op CC
    cost is latency-floor-dominated, so fewer+bigger is better), EXCEPT
    the last batch which ships four quarter AllGathers so its CC chain
    starts earlier and the serial tail is one quarter + one wo quarter.
  - host pre-shuffles x/weights to [128, ...] layouts so every DMA line
    is >= 8KB contiguous per partition; startup loads k-weights (first
    chunks) and x in pieces so the first matmul starts ~4us in.
  - RoPE multiplies read the projection PSUM directly (fp32), the
    add/sub pairs run in bf16 SBUF (DVE 4x mode).
  - fp8 DoubleRow for the projections was tried and REVERTED: weight
    quantization error (~3.6% RMS) passes straight through to the output
    (the attention average commutes through x @ dW), giving ~5.7e-2 rel
    err vs the 2e-2 gate, even though it ran at ~825us.
"""

import numpy as np
import ml_dtypes

B, S, D, H = 4, 2048, 2048, 16
HD = 128            # head dim
NCORES = 8
HPC = H // NCORES   # heads per core = 2
OSL = HPC * HD      # per-core o-slice = 256
ROWS = B * S        # 8192 flattened rows
DCH = D // 128      # 16 contraction chunks
SCH = 512           # seq chunk for projections
NBLK = ROWS // SCH  # 16 x column blocks
KB = S // 128       # 16 k-blocks per batch
QC = 512            # q chunk in attention
NQC = S // QC       # 4 quarters per batch
HB = S // 2         # half-batch column span of one AllGather unit
NU = B * 2          # 8 collective units (half batches)

BF16 = ml_dtypes.bfloat16
_NC_CACHE = None


def _build():
    import concourse.bass as bass  # noqa: F401
    import concourse.mybir as mybir
    import concourse.tile as tile
    from concourse import bacc

    fp32 = mybir.dt.float32
    bf16 = mybir.dt.bfloat16

    nc = bacc.Bacc(
        "TRN2",
        target_bir_lowering=False,
        debug=False,
        num_devices=NCORES,
    )

    # Pre-shuffled layouts (host): one contiguous >=8KB run per partition.
    xS = nc.declare_dram_parameter("xS", [128, NBLK, DCH, SCH], bf16,
                                   isOutput=False)
    wqS = nc.declare_dram_parameter("wqS", [128, 2, DCH, HD], bf16,
                                    isOutput=False)
    wkS = nc.declare_dram_parameter("wkS", [128, 2, DCH, HD], bf16,
                                    isOutput=False)
    wvS = nc.declare_dram_parameter("wvS", [128, DCH, OSL], bf16,
                                    isOutput=False)
    woS = nc.declare_dram_parameter("woS", [128, DCH, OSL], bf16,
                                    isOutput=False)
    cosd = nc.declare_dram_parameter("cosd", [128, S], bf16, isOutput=False)
    sind = nc.declare_dram_parameter("sind", [128, S], bf16, isOutput=False)
    outp = nc.declare_dram_parameter("out", [OSL, ROWS], fp32, isOutput=True)

    inv_sqrt_hd = 1.0 / float(np.sqrt(HD))

    with tile.TileContext(nc) as tc:
        with (
            tc.tile_pool(name="glob", bufs=1) as glob,
            tc.tile_pool(name="dram", bufs=1, space="DRAM") as dram,
            tc.tile_pool(name="qkv", bufs=2) as qkv,
            tc.tile_pool(name="xtp", bufs=2) as xtp,
            tc.tile_pool(name="attp", bufs=2) as attp,
            tc.tile_pool(name="treep", bufs=2) as treep,
            tc.tile_pool(name="ropep", bufs=2) as ropep,
            tc.tile_pool(name="rcpp", bufs=2) as rcpp,
            tc.tile_pool(name="atp", bufs=2) as atp,
            tc.tile_pool(name="gtp", bufs=2) as gtp,
            tc.tile_pool(name="otp", bufs=2) as otp,
            tc.tile_pool(name="psP", bufs=5, space="PSUM") as psP,
            tc.tile_pool(name="poP", bufs=2, space="PSUM") as poP,
            tc.tile_pool(name="psV", bufs=1, space="PSUM") as psV,
        ):
            ones128 = glob.tile([128, 128], bf16, name="ones128")
            nc.vector.memset(ones128[:], 1.0)

            wq_sb = glob.tile([128, 2, DCH, HD], bf16, name="wq_sb")
            wk_sb = glob.tile([128, 2, DCH, HD], bf16, name="wk_sb")
            wv_sb = glob.tile([128, DCH, OSL], bf16, name="wv_sb")
            wo_sb = glob.tile([128, DCH, OSL], bf16, name="wo_sb")
            cosb = glob.tile([128, S], bf16, name="cosb")
            sinb = glob.tile([128, S], bf16, name="sinb")
            # startup order matches the first chunks (k, then q, then v)
            nc.gpsimd.dma_start(wk_sb[:, 0, :, :], wkS[:, 0, :, :])
            xt00 = xtp.tile([128, DCH, SCH], bf16, name="xt00", tag="xt")
            for cq in range(4):
                nc.gpsimd.dma_start(xt00[:, cq * 4:(cq + 1) * 4, :],
                                    xS[:, 0, cq * 4:(cq + 1) * 4, :])
            nc.gpsimd.dma_start(wk_sb[:, 1, :, :], wkS[:, 1, :, :])
            nc.gpsimd.dma_start(wq_sb[:], wqS[:, :, :, :])
            nc.gpsimd.dma_start(wv_sb[:], wvS[:, :, :])
            nc.gpsimd.dma_start(cosb[:], cosd[:, :])
            nc.gpsimd.dma_start(sinb[:], sind[:, :])

            bounce = [dram.tile([OSL, HB], bf16, name=f"bounce{u}")
                      for u in range(NU)]
            gath = [dram.tile([NCORES * OSL, HB], bf16, addr_space="Shared",
                              name=f"gath{u}") for u in range(NU)]
            # the final batch ships as four quarter AllGathers so its
            # collectives start earlier and the serial tail is a quarter
            bounce_q = [dram.tile([OSL, QC], bf16, name=f"bounceq{j}")
                        for j in range(4)]
            gath_q = [dram.tile([NCORES * OSL, QC], bf16,
                                addr_space="Shared", name=f"gathq{j}")
                      for j in range(4)]

            def fetch_x(b, sc):
                xt = xtp.tile([128, DCH, SCH], bf16, name="xt", tag="xt")
                nc.gpsimd.dma_start(xt[:], xS[:, b * (S // SCH) + sc, :, :])
                return xt

            proj_result = {}

            def proj_gen(b, xt_first):
                """Generator issuing batch b's q/k/v projection, yielding
                after every matmul so the pump can weave it between the
                attention scores matmuls (hides the ScalarE exp chain)."""
                qt = qkv.tile([128, HPC, S], bf16, name="qt", tag="qt")
                kt = qkv.tile([128, HPC, S], bf16, name="kt", tag="kt")
                vt = qkv.tile([128, KB, HPC, HD], bf16, name="vt", tag="vt")
                proj_result[b] = (qt, kt, vt)
                xt, xt_next = xt_first, None

                for sc in range(S // SCH):
                    if sc > 0:
                        xt = xt_next
                    if sc + 1 < S // SCH:
                        xt_next = fetch_x(b, sc + 1)
                    sl = slice(sc * SCH, (sc + 1) * SCH)
                    cosr = cosb[:, sl]
                    sinr = sinb[:, sl]
                    def qk_chunk(w_sb, dstT, h):
                        ps = psP.tile([128, SCH], fp32, name="ps",
                                      tag="ps")
                        for c in range(DCH):
                            nc.tensor.matmul(
                                ps[:], w_sb[:, h, c, :],
                                xt[:, c, :],
                                start=(c == 0), stop=(c == DCH - 1))
                            yield
                        m1 = ropep.tile([128, SCH], bf16, name="m1",
                                        tag="m1")
                        m2 = ropep.tile([128, SCH], bf16, name="m2",
                                        tag="m2")
                        # m1 = [tr*cos ; ti*cos]; m2 swapped-halves =
                        # [ti*sin ; tr*sin] so the DVE add/sub uses
                        # equal SBUF base partitions (the PSUM operand
                        # may differ).
                        nc.vector.tensor_mul(m1[:], ps[:], cosr)
                        nc.vector.tensor_mul(
                            m2[0:64, :], ps[64:128, :], sinr[0:64, :])
                        nc.vector.tensor_mul(
                            m2[64:128, :], ps[0:64, :], sinr[64:128, :])
                        nc.vector.tensor_sub(
                            dstT[0:64, h, sl], m1[0:64, :], m2[0:64, :])
                        nc.vector.tensor_add(
                            dstT[64:128, h, sl], m2[64:128, :],
                            m1[64:128, :])

                    yield from qk_chunk(wk_sb, kt, 0)
                    yield from qk_chunk(wk_sb, kt, 1)
                    yield from qk_chunk(wq_sb, qt, 0)
                    yield from qk_chunk(wq_sb, qt, 1)
                    for pair in range(2):
                        psv = psV.tile([128, 2, OSL], fp32, name="psv")
                        for j in range(2):
                            ssb = pair * 2 + j
                            for c in range(DCH):
                                nc.tensor.matmul(
                                    psv[:, j, :],
                                    xt[:, c, ssb * 128:(ssb + 1) * 128],
                                    wv_sb[:, c, :],
                                    start=(c == 0), stop=(c == DCH - 1))
                                yield
                        kb0 = sc * 4 + pair * 2
                        nc.vector.tensor_copy(
                            vt[:, kb0:kb0 + 2, :, :],
                            psv[:].rearrange("p s (h d) -> p s h d", h=HPC))

            def allgather(src, dst):
                nc.gpsimd.collective_compute(
                    "AllGather",
                    mybir.AluOpType.bypass,
                    ins=[src.opt()],
                    outs=[dst.opt()],
                    replica_groups=[list(range(NCORES))],
                )

            def rb_pair(src, rc_):
                """Issue the two gather-readback DMAs for 512 columns."""
                gh = []
                for dh in range(2):
                    g = gtp.tile([128, DCH // 2, QC], bf16, name="gt",
                                 tag="gt")
                    nc.sync.dma_start(
                        g[:],
                        src[dh * 1024:(dh + 1) * 1024,
                            rc_ * QC:(rc_ + 1) * QC]
                        .rearrange("(c p) n -> p c n", p=128))
                    gh.append(g)
                return gh

            def wo_cols(src, col0, ncols):
                """Generator: wo for `ncols` gathered columns starting at
                output column col0, reading gather tile `src`."""
                for rc_ in range(ncols // QC):
                    gh = rb_pair(src, rc_)
                    for oc in range(OSL // 128):
                        psw = psP.tile([128, QC], fp32, name="psw", tag="ps")
                        for c in range(DCH):
                            nc.tensor.matmul(
                                psw[:],
                                wo_sb[:, c, oc * 128:(oc + 1) * 128],
                                gh[c // 8][:, c % 8, :],
                                start=(c == 0), stop=(c == DCH - 1))
                            yield
                        out_t = otp.tile([128, QC], fp32, name="out_t")
                        nc.vector.tensor_copy(out_t[:], psw[:])
                        cc = col0 + rc_ * QC
                        nc.sync.dma_start(
                            outp[oc * 128:(oc + 1) * 128, cc:cc + QC],
                            out_t[:])

            def wo_half_gen(b, half):
                return wo_cols(gath[b * 2 + half], b * S + half * HB, HB)

            def wo_quarter_gen(qc):
                return wo_cols(gath_q[qc], (B - 1) * S + qc * QC, QC)

            # one-unit flush lag: normalize/ship unit i-1 during unit i so
            # the psd matmul never waits on the DVE/ScalarE chains
            pend = []

            def flush_one():
                if not pend:
                    return
                b, h, qc, po, dsum = pend.pop(0)
                psd = psP.tile([128, QC], fp32, name="psd", tag="ps")
                nc.tensor.matmul(psd[:], ones128[:], dsum[:],
                                 start=True, stop=True)
                # 1/x as exp(-ln(x)) on ScalarE: the DVE reciprocal op
                # costs ~3.3us per [128,512] tile and would sit on the
                # critical path to the AllGather input
                lnd = rcpp.tile([128, QC], fp32, name="lnd", tag="lnd",
                                bufs=1)
                nc.scalar.activation(lnd[:], psd[:],
                                     mybir.ActivationFunctionType.Ln)
                rcp = rcpp.tile([128, QC], fp32, name="rcp", tag="rcp")
                nc.scalar.activation(rcp[:], lnd[:],
                                     mybir.ActivationFunctionType.Exp,
                                     scale=-1.0)
                a_t = atp.tile([128, QC], bf16, name="a_t")
                nc.vector.tensor_mul(a_t[:], po[:], rcp[:])
                if b == B - 1:
                    nc.gpsimd.dma_start(
                        bounce_q[qc][h * HD:(h + 1) * HD, :], a_t[:])
                    if h == 1:
                        allgather(bounce_q[qc], gath_q[qc])
                else:
                    u = b * 2 + qc // 2
                    col0 = (qc % 2) * QC
                    nc.gpsimd.dma_start(
                        bounce[u][h * HD:(h + 1) * HD, col0:col0 + QC],
                        a_t[:])
                    if h == 1 and qc % 2 == 1:
                        allgather(bounce[u], gath[u])

            # the micro-queue: generators yielding once per issued matmul;
            # pump(n) issues up to n matmuls of weave-filler work
            micro = []

            def pump(n):
                k = 0
                while k < n and micro:
                    try:
                        next(micro[0])
                        k += 1
                    except StopIteration:
                        micro.pop(0)

            PUMP_RATE = {0: 4, 1: 5, 2: 5, 3: 2}

            def attn_unit(b, qt, kt, vt, h, qc):
                expT = attp.tile([128, KB, QC], bf16, name="expT")
                for kb in range(KB):
                    pss = psP.tile([128, QC], fp32, name="pss", tag="ps")
                    nc.tensor.matmul(
                        pss[:],
                        kt[:, h, kb * 128:(kb + 1) * 128],
                        qt[:, h, qc * QC:(qc + 1) * QC],
                        start=True, stop=True)
                    nc.scalar.activation(
                        expT[:, kb, :], pss[:],
                        mybir.ActivationFunctionType.Exp,
                        scale=inv_sqrt_hd)
                    # weave filler matmuls so the scores stream never
                    # outruns the 5-deep pss rotation / exp chain
                    pump(PUMP_RATE[b])
                # denominator: tree-sum the 16 k-blocks on DVE (bf16 4x)
                s1 = treep.tile([128, 4, QC], bf16, name="s1", tag="s1")
                s2 = treep.tile([128, 4, QC], bf16, name="s2", tag="s2")
                s4 = treep.tile([128, 2, QC], bf16, name="s4", tag="s4")
                dsum = treep.tile([128, QC], bf16, name="dsum", tag="ds")
                nc.vector.tensor_add(s1[:], expT[:, 0:4, :], expT[:, 4:8, :])
                nc.vector.tensor_add(s2[:], expT[:, 8:12, :],
                                     expT[:, 12:16, :])
                nc.vector.tensor_add(s1[:], s1[:], s2[:])
                nc.vector.tensor_add(s4[:], s1[:, 0:2, :], s1[:, 2:4, :])
                nc.vector.tensor_add(dsum[:], s4[:, 0, :], s4[:, 1, :])
                # normalize/ship the previous unit (one-unit lag)
                flush_one()
                po = poP.tile([128, QC], fp32, name="po", tag="po")
                for kb in range(KB):
                    nc.tensor.matmul(
                        po[:], vt[:, kb, h, 0:HD], expT[:, kb, :],
                        start=(kb == 0), stop=(kb == KB - 1))
                pend.append((b, h, qc, po, dsum))

            # ---------------- main schedule ---------------------------
            # proj(0) runs unfilled upfront; proj(b+1) and wo(b-1)
            # quarters fill the attention(b) units.
            for _ in proj_gen(0, xt00):
                pass  # issue all of batch 0's projection now
            # wo weights are first needed ~300us in; load off the
            # startup critical path
            nc.gpsimd.dma_start(wo_sb[:], woS[:, :, :])

            for b in range(B):
                if b + 1 < B:
                    micro.append(proj_gen(b + 1, fetch_x(b + 1, 0)))
                qt, kt, vt = proj_result[b]
                # wo halves woven into batch b's units (appended to the
                # micro-queue at the given unit index); half1 of each
                # batch is deferred one extra batch to feed batch 3
                wo_sched = {}
                if b >= 2:
                    wo_sched[0] = [wo_half_gen(b - 2, 1)]
                if b >= 1:
                    wo_sched[2] = [wo_half_gen(b - 1, 0)]
                if b == B - 1:
                    wo_sched[4] = [wo_half_gen(b - 1, 1)]
                    # this batch's first quarter is gathered by unit 6
                    wo_sched[6] = [wo_quarter_gen(0)]
                units = [(h, qc) for qc in range(NQC) for h in range(HPC)]
                for ui, (h, qc) in enumerate(units):
                    micro.extend(wo_sched.get(ui, []))
                    attn_unit(b, qt, kt, vt, h, qc)
                if b + 1 < B:
                    # drain leftover weave work before the next batch
                    pump(10 ** 9)
            # tail: remaining quarters in AllGather-completion order
            flush_one()
            pump(10 ** 9)
            for qc in (1, 2, 3):
                for _ in wo_quarter_gen(qc):
                    pass

    nc.compile()
    return nc


def _shard_inputs(x, freqs_cos, freqs_sin, wq, wk, wv, wo):
    xf = np.asarray(x, dtype=np.float32).reshape(ROWS, D)
    xT = np.ascontiguousarray(xf.T).astype(BF16)  # [D, ROWS]
    # pre-shuffle: xS[p, blk, c, j] = xT[c*128+p, blk*512+j]
    xS = np.ascontiguousarray(
        xT.reshape(DCH, 128, NBLK, SCH).transpose(1, 2, 0, 3))
    fcT = np.asarray(freqs_cos, dtype=np.float32).T  # [64, S]
    fsT = np.asarray(freqs_sin, dtype=np.float32).T
    cosd = np.ascontiguousarray(np.concatenate([fcT, fcT], 0)).astype(BF16)
    sind = np.ascontiguousarray(np.concatenate([fsT, fsT], 0)).astype(BF16)
    # even indices (real half) then odd (imag half), per head
    perm = np.concatenate([np.arange(0, HD, 2), np.arange(1, HD, 2)])

    def shuf(wrows):  # [OSL, D] -> [128, DCH, OSL] bf16
        wT = np.ascontiguousarray(np.asarray(wrows, dtype=np.float32).T)
        return np.ascontiguousarray(
            wT.reshape(DCH, 128, OSL).transpose(1, 0, 2)).astype(BF16)

    def shufh(wrows):  # [OSL, D] -> [128, 2, DCH, HD] bf16 (head-major)
        wT = np.ascontiguousarray(np.asarray(wrows, dtype=np.float32).T)
        return np.ascontiguousarray(
            wT.reshape(DCH, 128, 2, HD).transpose(1, 2, 0, 3)).astype(BF16)

    in_maps = []
    for c in range(NCORES):
        rows = slice(c * OSL, (c + 1) * OSL)
        wq_c = np.asarray(wq)[rows].reshape(HPC, HD, D)[:, perm, :]
        wk_c = np.asarray(wk)[rows].reshape(HPC, HD, D)[:, perm, :]
        in_maps.append({
            "xS": xS,
            "wqS": shufh(wq_c.reshape(OSL, D)),
            "wkS": shufh(wk_c.reshape(OSL, D)),
            "wvS": shuf(np.asarray(wv)[rows]),
            "woS": shuf(np.asarray(wo)[rows]),
            "cosd": cosd,
            "sind": sind,
        })
    return in_maps


def run(inputs, trace=False, trace_cores=None):
    """Build (cached), run on 8 cores; returns (full_output, results)."""
    global _NC_CACHE
    from concourse.bass_utils import run_bass_kernel_spmd
    if _NC_CACHE is None:
        _NC_CACHE = _build()
    in_maps = _shard_inputs(**inputs)
    res = run_bass_kernel_spmd(
        _NC_CACHE, in_maps, core_ids=list(range(NCORES)), trace=trace,
        trace_cores=trace_cores)
    parts = [np.ascontiguousarray(
        np.asarray(res.results[c]["out"], dtype=np.float32).T)
        for c in range(NCORES)]
    full = np.concatenate(parts, axis=1).reshape(B, S, D)
    return full, res


def kernel(x, freqs_cos, freqs_sin, wq, wk, wv, wo):
    full, _ = run(dict(x=x, freqs_cos=freqs_cos, freqs_sin=freqs_sin,
                       wq=wq, wk=wk, wv=wv, wo=wo))
    return full
